# revision 11
# baseline (speedup 1.0000x reference)
"""Trainium2 Bass kernel for nn_Attn_30279519437427 (GQA attention block).

Sharding: 8 cores = batch(2) x kv_head(4). Each core computes 4 q-heads
(one GQA group) end-to-end: QKV projections, QK-RMSNorm + ortho-RoPE,
causal attention, per-head sigmoid gate, and its slice of the output
projection. The out-projection partial sums (one per core) are reduced
on the host per batch (standard tensor-parallel gather).

Layout strategy per core:
  - x is passed pre-transposed (xT: D x T) so projections run as
    out(t,d) = xT_chunk.T @ W_T_chunk with PE contraction over D.
  - q,k post-norm/rope are PE-transposed to (d, t) for the scores matmul.
  - scores are computed transposed (tk, t); exp'd (softmax max-subtraction
    is skipped: q,k are RMS-normalized so |score| <= sqrt(DH) = 11.3,
    safely inside fp32/exp range); the softmax denominator falls out of
    the attn@v matmul via an extra all-ones column appended to v.
  - y (t,d) is scaled per-partition by gate[t,h]/denom[t,h], transposed,
    and consumed as lhsT by the output projection.
Matmul dtypes: projections/scores/out-proj in float32r, attn-weights @ v
in bf16 (fp32 accumulation in PSUM everywhere).
"""
import sys

if "/opt/trn_rl_repo" not in sys.path:
    sys.path.insert(0, "/opt/trn_rl_repo")

import numpy as np

import concourse.bass as bass
import concourse.tile as tile
from concourse import bacc, mybir
from concourse.bass_utils import run_bass_kernel_spmd
from concourse.masks import make_identity

F32 = mybir.dt.float32
F32R = mybir.dt.float32r
BF16 = mybir.dt.bfloat16
AF = mybir.ActivationFunctionType

B, T, D = 2, 2048, 2048
H, HKV, DH = 16, 4, 128
HL = H // HKV  # q heads per core (GQA group)
EPS = 1e-6
N_CORES = 8
NTB = T // 128  # 16 t-blocks of 128
NDC = D // 128  # 16 D-chunks of 128
NJ = T // 512  # 4 t-groups of 512
VW = DH + 1  # v width with the ones column (denominator trick)
NEG = -1.0e30


def _emit(ctx, tc, xT, wqT, wkvT, v1s, cs, gwT, woT, out_p):
    nc = tc.nc

    const = ctx.enter_context(tc.tile_pool(name="const", bufs=1))
    xt_pool = ctx.enter_context(tc.tile_pool(name="xt", bufs=4))
    small = ctx.enter_context(tc.tile_pool(name="small", bufs=4))
    scratch = ctx.enter_context(tc.tile_pool(name="scratch", bufs=6))
    rope_tmp = ctx.enter_context(tc.tile_pool(name="ropetmp", bufs=8))
    exp_pool = ctx.enter_context(tc.tile_pool(name="exp", bufs=3))
    y_pool = ctx.enter_context(tc.tile_pool(name="ypool", bufs=4))
    yt_pool = ctx.enter_context(tc.tile_pool(name="ytpool", bufs=2))
    io_pool = ctx.enter_context(tc.tile_pool(name="io", bufs=4))
    psum = ctx.enter_context(tc.tile_pool(name="psum", bufs=2, space="PSUM"))

    # ---- resident weights / tables ----
    wq_sb = const.tile([128, NDC, 512], F32R, tag="wq")
    wkv_sb = const.tile([128, NDC, 256], F32R, tag="wkv")
    wo_sb = const.tile([128, HL, 2048], F32R, tag="wo")
    gw_sb = const.tile([16, 4], F32R, tag="gw")
    for dc in range(NDC):
        nc.sync.dma_start(out=wq_sb[:, dc, :], in_=wqT[dc * 128 : (dc + 1) * 128, :])
        nc.sync.dma_start(out=wkv_sb[:, dc, :], in_=wkvT[dc * 128 : (dc + 1) * 128, :])
    for h in range(HL):
        nc.sync.dma_start(out=wo_sb[:, h, :], in_=woT[h * 128 : (h + 1) * 128, :])
    nc.sync.dma_start(out=gw_sb[:, :], in_=gwT[:, :])

    ident = const.tile([128, 128], F32, tag="ident")
    make_identity(nc, ident)

    eps_q = const.tile([128, 1], F32, tag="eps_q")
    nc.vector.memset(eps_q, float(DH * EPS))
    eps_k = const.tile([128, 1], F32, tag="eps_k")
    nc.vector.memset(eps_k, float(EPS))

    # Sliding causal mask M[r, c] = 0 if (c - r) >= 384 else -1e30.
    # For a diagonal-region tk-block with sub-offset i0 (0..3), the slice
    # M[:, (3-i0)*128 : (3-i0)*128+512] is the additive mask for the
    # (tk=128, t=512) transposed-scores block.
    maskt = const.tile([128, 896], F32, tag="maskt")
    nc.gpsimd.memset(maskt, 0.0)
    nc.gpsimd.affine_select(
        out=maskt,
        in_=maskt,
        compare_op=mybir.AluOpType.is_ge,
        fill=NEG,
        base=-384,
        pattern=[[1, 896]],
        channel_multiplier=-1,
    )

    # ---- persistent per-core activations ----
    qT_sb = const.tile([128, HL, T], F32R, tag="qT")  # (d, h, t)
    kT_sb = const.tile([128, T], F32R, tag="kT")  # (d, t)
    v_sb = const.tile([128, NTB, VW], BF16, tag="v")  # (tk_in_blk, tkb, d+1)
    gates_sb = const.tile([128, NTB, HL], F32, tag="gates")
    for tb in range(NTB):
        nc.vector.memset(v_sb[:, tb, DH:VW], 1.0)

    # =======================================================
    # P1: projections + gate + rmsnorm + rope + transposes
    # =======================================================
    for tb in range(NTB):
        psq = psum.tile([128, 512], F32, tag="psA", bufs=2)
        pskv = psum.tile([128, 256], F32, tag="psB", bufs=1)
        for dc in range(NDC):
            xt = xt_pool.tile([128, 128], F32R, tag="xt")
            nc.sync.dma_start(
                out=xt,
                in_=xT[dc * 128 : (dc + 1) * 128, tb * 128 : (tb + 1) * 128],
            )
            nc.tensor.matmul(
                psq,
                lhsT=xt,
                rhs=wq_sb[:, dc, :],
                start=(dc == 0),
                stop=(dc == NDC - 1),
            )
            nc.tensor.matmul(
                pskv,
                lhsT=xt,
                rhs=wkv_sb[:, dc, :],
                start=(dc == 0),
                stop=(dc == NDC - 1),
            )
            if dc == 0:
                psg = psum.tile([128, 4], F32, tag="psC", bufs=1)
                nc.tensor.matmul(
                    psg, lhsT=xt[0:16, :], rhs=gw_sb, start=True, stop=True
                )
                nc.scalar.activation(gates_sb[:, tb, :], psg, AF.Sigmoid)

        # v = (x @ Wv'.T) + lamb*v1   (Wv pre-scaled by (1-lamb) on host)
        v1t = io_pool.tile([128, DH], F32, tag="v1t")
        nc.sync.dma_start(out=v1t, in_=v1s[tb * 128 : (tb + 1) * 128, :])
        nc.vector.tensor_add(v_sb[:, tb, 0:DH], pskv[:, 128:256], v1t)

        cst = io_pool.tile([128, DH], F32, tag="cst")
        nc.sync.dma_start(out=cst, in_=cs[tb * 128 : (tb + 1) * 128, :])

        # sum of squares per row for each of the 4 q heads + k
        ssq = small.tile([128, 5], F32, tag="ssq")
        for h in range(HL):
            sqt = scratch.tile([128, 128], F32, tag="sqt")
            nc.scalar.activation(
                sqt, psq[:, h * 128 : (h + 1) * 128], AF.Square,
                accum_out=ssq[:, h : h + 1],
            )
        sqk = scratch.tile([128, 128], F32, tag="sqt")
        nc.scalar.activation(sqk, pskv[:, 0:128], AF.Square, accum_out=ssq[:, 4:5])

        # rq = 1/sqrt(ssq + 128*eps) == DH^-0.5 / sqrt(mean+eps)  (score scale folded)
        # rk = 1/sqrt(ssq/128 + eps)
        rr = small.tile([128, 5], F32, tag="rr")
        nc.scalar.activation(rr[:, 0:4], ssq[:, 0:4], AF.Sqrt, bias=eps_q)
        nc.scalar.activation(
            rr[:, 4:5], ssq[:, 4:5], AF.Sqrt, scale=1.0 / DH, bias=eps_k
        )
        nc.vector.reciprocal(rr, rr)

        for idx in range(HL + 1):  # 0..3 = q heads, 4 = k
            src = psq[:, idx * 128 : (idx + 1) * 128] if idx < HL else pskv[:, 0:128]
            qs = scratch.tile([128, 128], F32, tag="qs")
            nc.scalar.activation(qs, src, AF.Copy, scale=rr[:, idx : idx + 1])
            # ortho rope: out[:, :64] = x0*cos - x1*sin ; out[:, 64:] = x1*cos + x0*sin
            qp = qs.rearrange("p (n two) -> p n two", two=2)
            x0 = qp[:, :, 0]
            x1 = qp[:, :, 1]
            cos = cst[:, 0:64]
            sin = cst[:, 64:128]
            qr = scratch.tile([128, 128], F32, tag="qr")
            ta = rope_tmp.tile([128, 64], F32, tag="ta")
            tb_ = rope_tmp.tile([128, 64], F32, tag="tb")
            nc.vector.tensor_mul(ta, x0, cos)
            nc.vector.tensor_mul(tb_, x1, sin)
            nc.vector.tensor_sub(qr[:, 0:64], ta, tb_)
            tc_ = rope_tmp.tile([128, 64], F32, tag="tc")
            td_ = rope_tmp.tile([128, 64], F32, tag="td")
            nc.vector.tensor_mul(tc_, x1, cos)
            nc.vector.tensor_mul(td_, x0, sin)
            nc.vector.tensor_add(qr[:, 64:128], tc_, td_)
            # transpose to (d, t)
            pt = psum.tile([128, 128], F32, tag="psC", bufs=1)
            nc.tensor.transpose(pt, qr, ident)
            dst = (
                qT_sb[:, idx, tb * 128 : (tb + 1) * 128]
                if idx < HL
                else kT_sb[:, tb * 128 : (tb + 1) * 128]
            )
            nc.vector.tensor_copy(dst, pt)

    # =======================================================
    # P2+P3: attention per (j, h), then out-proj per t-block
    # =======================================================
    for j in range(NJ):
        yts = [
            yt_pool.tile([128, HL, 128], F32R, tag=f"yt{i}", name=f"yt{i}")
            for i in range(4)
        ]
        for h in range(HL):
            nt = 4 * (j + 1)
            # one (y | denom) accumulator per t-sub-block, each in its OWN
            # psum bank: a matmul with start=True zeroes the whole 2KB bank
            # ("zero region"), so concurrent accumulation groups must never
            # share a bank
            psy = [
                psum.tile([128, VW], F32, tag=f"psY{i}", bufs=1, name=f"psy{i}")
                for i in range(4)
            ]
            for tkb in range(nt):
                pss = psum.tile([128, 512], F32, tag="psA", bufs=2)
                nc.tensor.matmul(
                    pss,
                    lhsT=kT_sb[:, tkb * 128 : (tkb + 1) * 128],
                    rhs=qT_sb[:, h, j * 512 : (j + 1) * 512],
                    start=True,
                    stop=True,
                )
                i0 = tkb - 4 * j
                if i0 >= 0:
                    nc.vector.tensor_add(
                        pss, pss, maskt[:, (3 - i0) * 128 : (3 - i0) * 128 + 512]
                    )
                ex = exp_pool.tile([128, 512], BF16, tag="ex")
                nc.scalar.activation(ex, pss, AF.Exp)
                for i in range(4):
                    if tkb > 4 * j + i:
                        continue  # block fully above the diagonal for this t-sub
                    nc.tensor.matmul(
                        psy[i],
                        lhsT=ex[:, i * 128 : (i + 1) * 128],
                        rhs=v_sb[:, tkb, :],
                        start=(tkb == 0),
                        stop=(tkb == 4 * j + i),
                    )
            for i in range(4):
                tb = 4 * j + i
                # c[t] = gate[t,h] / denom[t]
                rc = small.tile([128, 1], F32, tag="rc")
                nc.vector.reciprocal(rc, psy[i][:, DH:VW])
                cf = small.tile([128, 1], F32, tag="cf")
                nc.vector.tensor_mul(cf, rc, gates_sb[:, tb, h : h + 1])
                ysb = y_pool.tile([128, 128], F32, tag="ysb")
                nc.scalar.activation(ysb, psy[i][:, 0:DH], AF.Copy, scale=cf)
                pyt = psum.tile([128, 128], F32, tag="psC", bufs=1)
                nc.tensor.transpose(pyt, ysb, ident)
                nc.vector.tensor_copy(yts[i][:, h, :], pyt)

        for i in range(4):
            tb = 4 * j + i
            for nch in range(4):
                pso = psum.tile([128, 512], F32, tag="psA", bufs=2)
                for h in range(HL):
                    nc.tensor.matmul(
                        pso,
                        lhsT=yts[i][:, h, :],
                        rhs=wo_sb[:, h, nch * 512 : (nch + 1) * 512],
                        start=(h == 0),
                        stop=(h == HL - 1),
                    )
                osb = io_pool.tile([128, 512], F32, tag="osb")
                nc.scalar.copy(osb, pso)
                nc.sync.dma_start(
                    out=out_p[tb * 128 : (tb + 1) * 128, nch * 512 : (nch + 1) * 512],
                    in_=osb,
                )


def _build_program():
    nc = bacc.Bacc("TRN2", target_bir_lowering=False, debug=False, num_devices=N_CORES)
    ins = {
        "xT": nc.dram_tensor("xT", [D, T], F32R, kind="ExternalInput").ap(),
        "wqT": nc.dram_tensor("wqT", [D, HL * DH], F32R, kind="ExternalInput").ap(),
        "wkvT": nc.dram_tensor("wkvT", [D, 2 * DH], F32R, kind="ExternalInput").ap(),
        "v1s": nc.dram_tensor("v1s", [T, DH], F32, kind="ExternalInput").ap(),
        "cs": nc.dram_tensor("cs", [T, DH], F32, kind="ExternalInput").ap(),
        "gwT": nc.dram_tensor("gwT", [16, HL], F32R, kind="ExternalInput").ap(),
        "woT": nc.dram_tensor("woT", [HL * DH, D], F32R, kind="ExternalInput").ap(),
    }
    out_p = nc.dram_tensor("out_p", [T, D], F32, kind="ExternalOutput").ap()
    from contextlib import ExitStack

    with tile.TileContext(nc) as tc, ExitStack() as ctx:
        _emit(ctx, tc, out_p=out_p, **ins)
    nc.compile()
    return nc


_NC_CACHE = None


def kernel(x, pos_ids, cos, sin, v1, Wq, Wk, Wv, Wo, gate_w, v_lamb):
    global _NC_CACHE
    x = np.asarray(x, dtype=np.float32)
    cos = np.asarray(cos, dtype=np.float32)
    sin = np.asarray(sin, dtype=np.float32)
    v1 = np.asarray(v1, dtype=np.float32)
    Wq = np.asarray(Wq, dtype=np.float32)
    Wk = np.asarray(Wk, dtype=np.float32)
    Wv = np.asarray(Wv, dtype=np.float32)
    Wo = np.asarray(Wo, dtype=np.float32)
    gate_w = np.asarray(gate_w, dtype=np.float32)
    lamb = float(np.asarray(v_lamb))

    cs_full = np.ascontiguousarray(np.concatenate([cos, sin], axis=1))  # (T, 128)
    in_maps = []
    for c in range(N_CORES):
        b, k = divmod(c, HKV)
        hs = k * HL  # first q head of this core's GQA group
        wq_s = Wq[hs * DH : (hs + HL) * DH, :]  # (512, D)
        wk_s = Wk[k * DH : (k + 1) * DH, :]  # (128, D)
        wv_s = Wv[k * DH : (k + 1) * DH, :] * (1.0 - lamb)
        in_maps.append(
            {
                "xT": np.ascontiguousarray(x[b].T),
                "wqT": np.ascontiguousarray(wq_s.T),
                "wkvT": np.ascontiguousarray(np.concatenate([wk_s, wv_s], 0).T),
                "v1s": np.ascontiguousarray(v1[b, k] * lamb),
                "cs": cs_full,
                "gwT": np.ascontiguousarray(gate_w[hs : hs + HL, :].T),
                "woT": np.ascontiguousarray(Wo[:, hs * DH : (hs + HL) * DH].T),
            }
        )

    if _NC_CACHE is None:
        _NC_CACHE = _build_program()
    res = run_bass_kernel_spmd(_NC_CACHE, in_maps, core_ids=list(range(N_CORES)))

    out = np.zeros((B, T, D), dtype=np.float32)
    for c in range(N_CORES):
        b = c // HKV
        out[b] += res.results[c]["out_p"]
    return out, v1
